# revision 20
# baseline (speedup 1.0000x reference)
"""DTCRF loss (nn_DTCRF_13091060318392) — Trainium2 Bass kernel, 8 NeuronCores.

Self-contained: takes FULL inputs (B=512, S=2048, N=49), shards the batch over
8 cores (64 rows each), returns the scalar loss.

Algorithm (denominator): the CRF forward recurrence z_t = (E^T z_{t-1}) * e_t
is linear in z, so the product of per-step positive matrices over a long time
segment is numerically rank-1 (Perron-Frobenius).  Time splits into a host
fp64 prefix (57 steps), 28 device segments of L=69, and a host fp64 suffix
(58 steps).  Each device segment k contributes a full forward image
r_k = P_k @ start and a TRUNCATED backward probe s~_k: only the first w=32
steps at the segment's low end determine the direction of P_k^T 1 (Perron
contraction), and the scale-invariant telescoping

  den = ln(s_last . r_K) + sum_k [ ln(s~_k . r_{k-1}) - ln(s~_k . 1) ]

tolerates arbitrary probe scale.  Forward chains pack two segments per
98-partition instruction stream (weights blockdiag(aE, aE) — one 64-col
matmul drives both); probes pack likewise with blockdiag(aE^T, aE^T) and run
at half cadence for the first 64 iterations.  A constant a = exp(-mean log
growth) folded into the weights keeps magnitudes in range with no runtime
rescaling.  Engine plan per step (walrus forbids GPSIMD touching PSUM; DVE
pays a 120-cycle PSUM bubble per instruction, so chains fuse in groups of
6-8): forward groups PE matmul -> fused DVE mul (PSUM); probe groups
PE matmul -> fused ACT copy -> fused DVE bf16 mul.  The numerator
(transition + emission gathers) is computed on host in fp64.
"""

import sys
import types
from contextlib import ExitStack

import numpy as np

# ---------------------------------------------------------------------------
# environment shims (NTFF profile hook absent in this image; walrus here
# supports at most one sync wait per instruction)
# ---------------------------------------------------------------------------


def _apply_ntff_shim():
    if "antenv.axon_hooks" not in sys.modules:
        mod = types.ModuleType("antenv.axon_hooks")
        mod._hook = None
        mod.set_axon_ntff_profile_hook = lambda h: setattr(mod, "_hook", h)
        mod.get_axon_ntff_profile_hook = lambda: mod._hook
        sys.modules["antenv.axon_hooks"] = mod
        try:
            import antenv

            antenv.axon_hooks = mod
        except ImportError:
            pass
    try:
        from antenv.axon_hooks import (
            get_axon_ntff_profile_hook,
            set_axon_ntff_profile_hook,
        )

        if get_axon_ntff_profile_hook() is None:
            from trn_agent_boot.trn_boot import _ntff_profile_via_ctypes

            set_axon_ntff_profile_hook(
                _ntff_profile_via_ctypes("/opt/axon/libaxon_pjrt.so")
            )
    except Exception:
        pass
    try:
        import concourse.bass_utils as bu

        bu.upload_artifacts = lambda tmpdir: f"file://{tmpdir}"
    except Exception:
        pass


def _split_multiwaits(nc):
    import bass_rust
    from concourse import mybir

    for bassbb in nc.bb_map.values():
        bb = bassbb.bb
        new = []
        changed = False
        for inst in bb.instructions:
            si = inst.sync_info
            waits = list(si.on_wait) if si and si.on_wait else []
            if len(waits) > 1:
                changed = True
                for k, w in enumerate(waits[:-1]):
                    nop = mybir.InstNoOp(name=f"{inst.name}_wsplit{k}", ins=[], outs=[])
                    nop.engine = inst.engine
                    nop.sync_info = bass_rust.SyncInfo(on_wait=[w], on_update=[])
                    try:
                        nc.register_instruction(nop)
                    except Exception:
                        pass
                    new.append(nop)
                si.on_wait = [waits[-1]]
                inst.sync_info = si
            new.append(inst)
        if changed:
            bb.instructions = new


# ---------------------------------------------------------------------------
# constants
# ---------------------------------------------------------------------------

N = 49          # tags
NB = 98         # two 49-partition chain blocks per instruction stream
BPC = 64        # batch rows per core
NCORES = 8
S = 2048
H1 = 63         # host fp64 prefix steps
H2 = 64         # host fp64 suffix steps
KF = 30         # device forward segments
L_SEG = 64      # forward segment length
W_PR = 24       # backward probe length
NST = KF // 2   # 15 packed streams (fwd) and 15 (probes)

assert H1 + KF * L_SEG + H2 == S - 1
BOUNDS = [H1 + L_SEG * i for i in range(KF + 1)]  # fwd seg k: (BOUNDS[k-1], BOUNDS[k]]
# probe list: segs 2..KF plus one duplicate to fill 2*NST slots
PROBES = list(range(2, KF + 1)) + [KF]
assert len(PROBES) == 2 * NST

# group sizes (streams per fused instruction group); psum bank = 512 fp32
FGRP = [8, 7]   # forward groups -> direct DVE psum mul
PGRP = [8, 7]   # probe groups  -> ACT copy + DVE bf16 mul

FCH = [4, 12, 12, 12, 12, 12]       # esf chunking along j (sum = 64; small
                                    # first chunk so compute starts early)
PCH = [4, 10, 10]                   # esp chunking along pj (sum = 24)
assert sum(FCH) == L_SEG and sum(PCH) == W_PR

_NC_CACHE = {}


def _build_nc():
    import concourse.bass as bass
    import concourse.tile as tile
    from concourse import mybir

    F32 = mybir.dt.float32
    BF16 = mybir.dt.bfloat16

    nc = bass.Bass()
    esf_d = nc.dram_tensor("esf", [NB, L_SEG, NST, BPC], BF16, kind="ExternalInput")
    esp_d = nc.dram_tensor("esp", [NB, W_PR, NST, BPC], BF16, kind="ExternalInput")
    wff_d = nc.dram_tensor("wff", [NB, NB], BF16, kind="ExternalInput")
    wbb_d = nc.dram_tensor("wbb", [NB, NB], BF16, kind="ExternalInput")
    zif_d = nc.dram_tensor("zif", [NB, NST, BPC], BF16, kind="ExternalInput")
    zip_d = nc.dram_tensor("zip", [NB, NST, BPC], BF16, kind="ExternalInput")
    zof_d = nc.dram_tensor("zof", [NB, NST, BPC], BF16, kind="ExternalOutput")
    zop_d = nc.dram_tensor("zop", [NB, NST, BPC], BF16, kind="ExternalOutput")

    with tile.TileContext(nc) as tc, ExitStack() as ctx:
        singles = ctx.enter_context(tc.tile_pool(name="singles", bufs=1))
        esfp = ctx.enter_context(tc.tile_pool(name="esfp", bufs=2))
        espp = ctx.enter_context(tc.tile_pool(name="espp", bufs=2))
        zp = ctx.enter_context(tc.tile_pool(name="zp", bufs=2))
        ucp = ctx.enter_context(tc.tile_pool(name="ucp", bufs=2))
        up = ctx.enter_context(tc.tile_pool(name="up", bufs=1, space="PSUM"))

        wff_s = singles.tile([NB, NB], BF16)
        nc.sync.dma_start(out=wff_s, in_=wff_d[:])
        wbb_s = singles.tile([NB, NB], BF16)
        nc.sync.dma_start(out=wbb_s, in_=wbb_d[:])

        fgo = np.cumsum([0] + FGRP)
        pgo = np.cumsum([0] + PGRP)
        zf = zp.tile([NB, NST, BPC], BF16, tag="zf")
        nc.sync.dma_start(out=zf, in_=zif_d[:])
        zq = zp.tile([NB, NST, BPC], BF16, tag="zq")
        nc.sync.dma_start(out=zq, in_=zip_d[:])

        f_tiles = [None] * len(FCH)
        p_tiles = [None] * len(PCH)
        fco = np.cumsum([0] + FCH)
        pco = np.cumsum([0] + PCH)

        def load_f(ci):
            t = esfp.tile([NB, FCH[ci], NST, BPC], BF16, tag="esf")
            nc.sync.dma_start(out=t, in_=esf_d[:, fco[ci] : fco[ci + 1], :, :])
            f_tiles[ci] = t

        def load_p(ci):
            t = espp.tile([NB, PCH[ci], NST, BPC], BF16, tag="esp")
            nc.sync.dma_start(out=t, in_=esp_d[:, pco[ci] : pco[ci + 1], :, :])
            p_tiles[ci] = t

        load_f(0)
        load_p(0)
        fci = pci = 0
        for j in range(L_SEG):
            if j >= fco[fci + 1]:
                fci += 1
            if fci + 1 < len(FCH) and j == fco[fci]:
                load_f(fci + 1)
            es_f = f_tiles[fci]
            # one 2-bank PSUM tile; each matmul's slice is bank-aligned
            # (first group is 8 chains = exactly 2KB/partition)
            u = up.tile([NB, 16, BPC], F32, tag="uf")
            for g in range(len(FGRP)):
                nc.tensor.matmul(
                    u[:, fgo[g] : fgo[g + 1], :],
                    wff_s,
                    zf[:, fgo[g] : fgo[g + 1], :],
                    start=True,
                    stop=True,
                )
            znxt = zp.tile([NB, NST, BPC], BF16, tag="zf")
            nc.vector.tensor_mul(znxt, u[:, :NST, :], es_f[:, j - fco[fci], :, :])
            zf = znxt
            if j % 2 == 0 and j // 2 < W_PR:
                pj = j // 2
                if pj >= pco[pci + 1]:
                    pci += 1
                if pci + 1 < len(PCH) and pj == pco[pci]:
                    load_p(pci + 1)
                es_p = p_tiles[pci]
                u = up.tile([NB, 16, BPC], F32, tag="uq")
                for g in range(len(PGRP)):
                    nc.tensor.matmul(
                        u[:, pgo[g] : pgo[g + 1], :],
                        wbb_s,
                        zq[:, pgo[g] : pgo[g + 1], :],
                        start=True,
                        stop=True,
                    )
                uc = ucp.tile([NB, NST, BPC], BF16, tag="uc")
                nc.scalar.copy(out=uc, in_=u[:, :NST, :])
                znxt = zp.tile([NB, NST, BPC], BF16, tag="zq")
                nc.vector.tensor_mul(znxt, uc, es_p[:, pj - pco[pci], :, :])
                zq = znxt
                if pj == W_PR - 1:
                    nc.sync.dma_start(out=zop_d[:], in_=zq)

        nc.sync.dma_start(out=zof_d[:], in_=zf)

    _split_multiwaits(nc)
    return nc


# ---------------------------------------------------------------------------
# host math
# ---------------------------------------------------------------------------


def _build_transitions_np(p_in, p_cross, p_out, p_to_out, p_from_out):
    E, M = 12, 4
    eye = np.eye(E, dtype=bool)
    blocks = np.where(eye[:, :, None, None], p_in, p_cross)
    inner = blocks.transpose(0, 2, 1, 3).reshape(E * M, E * M)
    T = np.zeros((N, N), dtype=np.float32)
    T[1:, 1:] = inner
    T[0, 0] = p_out[0]
    T[0, 1:] = np.tile(p_from_out, E)
    T[1:, 0] = np.tile(p_to_out, E)
    return T


def _ref_numpy_general(inputs, tags, mask, T):
    """Slow but general fallback (used only if mask is not all ones)."""
    B, S_, _ = inputs.shape
    Tf = T.astype(np.float64)
    lg = inputs.astype(np.float64)
    alpha = lg[:, 0, :]
    for t in range(1, S_):
        inner = alpha[:, :, None] + Tf[None, :, :] + lg[:, t, None, :]
        m = inner.max(axis=1, keepdims=True)
        new_alpha = np.log(np.exp(inner - m).sum(axis=1)) + m[:, 0, :]
        alpha = np.where((mask[:, t] > 0)[:, None], new_alpha, alpha)
    am = alpha.max(1)
    den = np.log(np.exp(alpha - am[:, None]).sum(1)) + am
    fm = mask.astype(np.float64)
    tg = tags.astype(np.int64)
    trans = (Tf[tg[:, :-1], tg[:, 1:]] * fm[:, 1:]).sum(1)
    emit = (
        np.take_along_axis(lg[:, :-1, :], tg[:, :-1, None], axis=2)[:, :, 0]
        * fm[:, :-1]
    ).sum(1)
    last_idx = mask.sum(1).astype(np.int64) - 1
    last_tags = np.take_along_axis(tg, last_idx[:, None], axis=1)[:, 0]
    last_emit = lg[np.arange(B), -1, last_tags]
    num = trans + emit + last_emit * fm[:, -1]
    return np.float32(np.sum(num - den))


def _host_ends(lg, E64):
    """fp64 prefix z_H1 (from z0, H1 steps) and exact suffix s_last^T =
    1^T P_suffix (H2 steps), both max-normalized with accumulated logs."""
    B = lg.shape[0]
    e = np.exp(lg[:, : H1 + 1, :].astype(np.float64))
    z = e[:, 0, :]
    ln_pre = np.zeros(B)
    for t in range(1, H1 + 1):
        z = (z @ E64) * e[:, t, :]
        m = z.max(1, keepdims=True)
        ln_pre += np.log(m[:, 0])
        z /= m
    e2 = np.exp(lg[:, BOUNDS[KF] + 1 :, :].astype(np.float64))
    y = np.ones((B, N))
    ln_suf = np.zeros(B)
    for i in range(H2):  # t = S-1 down to BOUNDS[KF]+1
        y = (y * e2[:, H2 - 1 - i, :]) @ E64.T
        m = y.max(1, keepdims=True)
        ln_suf += np.log(m[:, 0])
        y /= m
    return z, ln_pre, y, ln_suf


def _estimate_alpha(lg, E64):
    z = np.ones((4, N))
    e = np.exp(lg[:4, H1 + 1 : H1 + 129, :].astype(np.float64))
    acc = 0.0
    for t in range(128):
        z = (z @ E64) * e[:, t, :]
        m = z.max(1, keepdims=True)
        acc += np.log(m[:, 0]).mean()
        z /= m
    return float(np.exp(-acc / 128.0))


def _prepare(inputs, tags, T):
    """Build device input maps + host context. inputs: (B,S,N) fp32."""
    import ml_dtypes

    BF = ml_dtypes.bfloat16
    E64 = np.exp(T.astype(np.float64))

    alpha = _estimate_alpha(inputs, E64)
    wff = np.zeros((NB, NB), np.float32)
    wff[:N, :N] = alpha * E64
    wff[N:, N:] = alpha * E64
    wbb = np.zeros((NB, NB), np.float32)
    wbb[:N, :N] = alpha * E64.T
    wbb[N:, N:] = alpha * E64.T
    wff = wff.astype(BF)
    wbb = wbb.astype(BF)

    z_h1, ln_pre, s_last, ln_suf = _host_ends(inputs, E64)
    ee16 = np.exp(inputs).astype(BF)  # (B, S, N)
    zh1_16 = z_h1.astype(BF)

    in_maps = []
    for cc in range(NCORES):
        b0 = cc * BPC
        blk = ee16[b0 : b0 + BPC]  # (64, S, 49)
        esf = np.empty((NB, L_SEG, NST, BPC), BF)
        esp = np.empty((NB, W_PR, NST, BPC), BF)
        zif = np.empty((NB, NST, BPC), BF)
        zip_ = np.empty((NB, NST, BPC), BF)
        for i in range(NST):
            for half, k in ((0, 2 * i + 1), (1, 2 * i + 2)):
                lo, hi = BOUNDS[k - 1], BOUNDS[k]
                sl = slice(half * N, half * N + N)
                esf[sl, :, i, :] = blk[:, lo + 1 : hi + 1, :].transpose(2, 1, 0)
                if k == 1:
                    zif[sl, i, :] = zh1_16[b0 : b0 + BPC].T
                else:
                    zif[sl, i, :] = np.float32(1.0)
            for half in (0, 1):
                k = PROBES[2 * i + half]
                lo = BOUNDS[k - 1]
                sl = slice(half * N, half * N + N)
                # probe slots pj: e_{lo+W-1-pj} for pj < W-1, ones at pj=W-1
                esp[sl, : W_PR - 1, i, :] = blk[
                    :, lo + W_PR - 1 : lo : -1, :
                ].transpose(2, 1, 0)
                esp[sl, W_PR - 1, i, :] = np.float32(1.0)
                zip_[sl, i, :] = blk[:, lo + W_PR, :].T
        in_maps.append(
            {"esf": esf, "esp": esp, "wff": wff, "wbb": wbb, "zif": zif, "zip": zip_}
        )

    ctx = {
        "alpha": alpha,
        "ln_pre": ln_pre,
        "s_last": s_last,
        "ln_suf": ln_suf,
    }
    return in_maps, ctx


def _den_from_results(results, ctx):
    """Assemble den (B,) in fp64 from per-core zof/zop tensors."""
    lam = np.log(ctx["alpha"])
    den = ctx["ln_pre"] + ctx["ln_suf"]
    for cc in range(NCORES):
        b0 = cc * BPC
        rb = slice(b0, b0 + BPC)
        zof = np.asarray(results[cc]["zof"]).astype(np.float64)  # (98, 14, 64)
        zop = np.asarray(results[cc]["zop"]).astype(np.float64)
        r = {}
        st = {}
        for i in range(NST):
            r[2 * i + 1] = zof[:N, i, :].T  # (64, 49)
            r[2 * i + 2] = zof[N:, i, :].T
            st[PROBES[2 * i]] = zop[:N, i, :].T
            st[PROBES[2 * i + 1]] = zop[N:, i, :].T
        d = (
            np.log(np.einsum("bn,bn->b", ctx["s_last"][rb], r[KF]))
            - L_SEG * lam
        )
        for k in range(2, KF + 1):
            d += (
                np.log(np.einsum("bn,bn->b", st[k], r[k - 1]))
                - np.log(st[k].sum(1))
                - L_SEG * lam
            )
        den[rb] += d
    return den


def kernel(inputs, tags, mask, p_in, p_cross, p_out, p_to_out, p_from_out):
    B, S_, Nn = inputs.shape
    T = _build_transitions_np(
        np.asarray(p_in, np.float32),
        np.asarray(p_cross, np.float32),
        np.asarray(p_out, np.float32),
        np.asarray(p_to_out, np.float32),
        np.asarray(p_from_out, np.float32),
    )
    inputs = np.asarray(inputs, dtype=np.float32)
    tags64 = np.asarray(tags).astype(np.int64)

    if not np.all(np.asarray(mask) == 1) or S_ != S or B != NCORES * BPC:
        return _ref_numpy_general(inputs, tags64, np.asarray(mask), T)

    _apply_ntff_shim()
    from concourse.bass_utils import run_bass_kernel_spmd

    if "nc" not in _NC_CACHE:
        _NC_CACHE["nc"] = _build_nc()
    nc = _NC_CACHE["nc"]

    in_maps, ctx = _prepare(inputs, tags64, T)
    res = run_bass_kernel_spmd(nc, in_maps, core_ids=list(range(NCORES)))
    den = _den_from_results(res.results, ctx)

    # numerator on host (mask is all ones)
    Tf = T.astype(np.float64)
    trans = Tf[tags64[:, :-1], tags64[:, 1:]].sum(axis=1)
    emit = np.take_along_axis(
        inputs, tags64[:, :, None].astype(np.int64), axis=2
    )[:, :, 0].astype(np.float64).sum(axis=1)
    num = trans + emit

    return np.float32(np.sum(num - den))
